# revision 1
# baseline (speedup 1.0000x reference)
"""Causal self-attention (GPT-style block) on 8 Trainium2 NeuronCores.

Problem: x[4,2048,1024] -> qkv = x@W_attn+b ; 16-head causal attention
(head_dim 64) ; out = y@W_proj+b_proj.

Sharding: tensor-parallel over heads. Core c owns heads {2c, 2c+1}:
  - computes q^T/k^T/v^T for its heads over the full batch via matmuls
    against a host-pretransposed x^T (bf16),
  - runs causal attention for its 8 (batch, head) pairs entirely in SBUF
    (S^T layout: scores tile [128 j, 512 i]; exp on ScalarE, causal mask on
    GpSimd, PV matmul with a ones-column appended to V producing both
    y_raw^T and the softmax denominator in one PSUM tile),
  - AllToAll swaps the head dim for the row dim, after which each core
    computes the output projection for its 1024 rows (pure gather on host).

Numerics: bf16 operands with fp32 PSUM accumulation everywhere; softmax
skips the max-subtraction (scores are O(1) by construction; exp stays
finite) which matches the reference to ~1e-5 in fp32.
"""

import numpy as np
import ml_dtypes
from contextlib import ExitStack

import concourse.bass as bass
import concourse.tile as tile
from concourse import bacc, mybir
from concourse.tile_rust import add_dep_helper
from concourse.bass_utils import run_bass_kernel_spmd

F32 = mybir.dt.float32
BF16 = mybir.dt.bfloat16
AF = mybir.ActivationFunctionType

N_CORES = 8
B, T, C, H = 4, 2048, 1024, 16
HD = C // H            # 64 head dim
HPC = H // N_CORES     # 2 heads per core
FPC = HPC * HD         # 128 features per core
BT = B * T             # 8192 rows
TCHUNK = 512           # t chunk in qkv phase
NT_CHUNKS = BT // TCHUNK
QB = 512               # query block
NQB = T // QB          # 4 per batch
JTN = T // 128         # 16 j-tiles per batch
ROWS = BT // N_CORES   # 1024 rows per core after AllToAll
KC = C // 128          # 8 contraction tiles over C
VW = 80                # [V | 1] row unit; DMA-transpose dest needs 32B-aligned offsets
STRIPE = ROWS // B     # 256 rows per (core, batch) after the striped AllToAll
SCALE = 1.0 / np.sqrt(HD)

LAST_RESULTS = None    # test.py reads exec_time_ns off this


def build_program(nc, debug=False):
    xT = nc.dram_tensor("xT", [C, BT], BF16, kind="ExternalInput").ap()
    wq = nc.dram_tensor("wq", [C, FPC], BF16, kind="ExternalInput").ap()
    wk = nc.dram_tensor("wk", [C, FPC], BF16, kind="ExternalInput").ap()
    wv = nc.dram_tensor("wv", [C, FPC], BF16, kind="ExternalInput").ap()
    bqkv = nc.dram_tensor("bqkv", [3, FPC], F32, kind="ExternalInput").ap()
    wp = nc.dram_tensor("wp", [C, C], BF16, kind="ExternalInput").ap()
    bp = nc.dram_tensor("bp", [C], F32, kind="ExternalInput").ap()
    out = nc.dram_tensor("out", [ROWS, C], F32, kind="ExternalOutput").ap()
    cc_in = nc.dram_tensor("cc_in", [N_CORES, FPC, ROWS], BF16, kind="Internal").ap()
    cc_out = nc.dram_tensor("cc_out", [N_CORES, FPC, ROWS], BF16, kind="Internal").ap()
    vT_dram = nc.dram_tensor("vT_dram", [FPC, BT], BF16, kind="Internal").ap()

    dbg = None
    if debug:
        dbg = {
            "d_qT": nc.dram_tensor("d_qT", [128, BT], BF16, kind="ExternalOutput").ap(),
            "d_kT": nc.dram_tensor("d_kT", [128, BT], BF16, kind="ExternalOutput").ap(),
            "d_vT": nc.dram_tensor("d_vT", [128, BT], BF16, kind="ExternalOutput").ap(),
            "d_vsb": nc.dram_tensor("d_vsb", [128, B * JTN, HPC, VW], BF16, kind="ExternalOutput").ap(),
            "d_yT0": nc.dram_tensor("d_yT0", [64, BT], BF16, kind="ExternalOutput").ap(),
            "d_yT1": nc.dram_tensor("d_yT1", [64, BT], BF16, kind="ExternalOutput").ap(),
        }
    with tile.TileContext(nc) as tc:
        with ExitStack() as ctx:
            emit(ctx, tc, xT, wq, wk, wv, bqkv, wp, bp, out, cc_in, cc_out, vT_dram, dbg)
    return nc


def emit(ctx, tc, xT, wq, wk, wv, bqkv, wp, bp, out, cc_in, cc_out, vT_dram, dbg=None):
    nc = tc.nc
    res = ctx.enter_context(tc.tile_pool(name="resident", bufs=1))

    # ---------- resident SBUF ----------
    qT = res.tile([128, BT], BF16)
    kT = res.tile([128, BT], BF16)
    vsb = res.tile([128, B * JTN, HPC, VW], BF16)     # [V | 1 | pad] per j-tile/head
    wq_sb = res.tile([128, KC, FPC], BF16)
    wk_sb = res.tile([128, KC, FPC], BF16)
    wv_sb = res.tile([128, KC, FPC], BF16)
    b_sb = res.tile([128, 3], F32)
    wp_sb = res.tile([128, KC, C], BF16)
    bp_sb = res.tile([128, C], F32)
    yT0 = res.tile([64, BT], BF16)
    yT1 = res.tile([64, BT], BF16)

    # ---------- constant/weight loads (wp/bp deferred to phase 3) ----------
    nc.sync.dma_start(wq_sb[:], wq.rearrange("(a p) m -> p a m", p=128))
    nc.sync.dma_start(wk_sb[:], wk.rearrange("(a p) m -> p a m", p=128))
    nc.sync.dma_start(wv_sb[:], wv.rearrange("(a p) m -> p a m", p=128))
    nc.sync.dma_start(b_sb[:], bqkv.rearrange("b p -> p b"))
    nc.vector.memset(vsb[:, :, :, HD : HD + 1], 1.0)

    # ---------- phase 1: qkv projections (q^T, k^T, v^T) ----------
    transpose_insts = []
    ph12 = ExitStack()
    xpool = ph12.enter_context(tc.tile_pool(name="xt", bufs=3))
    qkvps = ph12.enter_context(tc.tile_pool(name="qkvps", bufs=2, space="PSUM"))
    xT_t = xT.rearrange("(a p) t -> p a t", p=128)
    for tci in range(NT_CHUNKS):
        t0 = tci * TCHUNK
        xt = xpool.tile([128, KC, TCHUNK], BF16, tag="xt")
        # split the 1 MiB chunk load across 4 DMA queues
        for spl in range(4):
            nc.sync.dma_start(
                xt[:, 2 * spl : 2 * spl + 2, :],
                xT_t[:, 2 * spl : 2 * spl + 2, t0 : t0 + TCHUNK],
            )
        for w_sb, bi, dst in ((wq_sb, 0, qT), (wk_sb, 1, kT), (wv_sb, 2, None)):
            ps = qkvps.tile([128, TCHUNK], F32, tag="qkvps")
            for a in range(KC):
                nc.tensor.matmul(
                    ps[:], lhsT=w_sb[:, a, :], rhs=xt[:, a, :],
                    start=(a == 0), stop=(a == KC - 1),
                )
            # evictions on DVE: keeps ScalarE exp-only (no ACT table switches)
            if dst is not None:
                nc.vector.tensor_scalar_add(
                    dst[:, t0 : t0 + TCHUNK], ps[:], b_sb[:, bi : bi + 1]
                )
            else:
                vb = xpool.tile([128, TCHUNK], BF16, tag="vbounce")
                nc.vector.tensor_scalar_add(vb[:], ps[:], b_sb[:, bi : bi + 1])
                nc.sync.dma_start(vT_dram[:, t0 : t0 + TCHUNK], vb[:])
        # v -> natural layout for this chunk's 4 j-tiles (DRAM->SBUF DMA
        # transpose; dest offsets 32B-aligned).  Split across the two HWDGE
        # engines: these cost ~1.2us of engine time each.
        # all transposes on ONE engine: concurrent xbar-mode flips from two
        # HWDGE engines (sync DMAs + scalar transposes) corrupted data
        # nondeterministically on HW.
        for g in range(4 * tci, 4 * tci + 4):
            for h in range(HPC):
                ti = nc.sync.dma_start_transpose(
                    vsb[:, g, h, 0:HD],
                    vT_dram[h * HD : (h + 1) * HD, g * 128 : (g + 1) * 128],
                )
                transpose_insts.append(ti)

    ph12.close()  # release phase-1 PSUM/xt pools before the attention pools open

    nc.sync.dma_start(wp_sb[:], wp.rearrange("(a p) e -> p a e", p=128))
    bp_bcast = bass.AP(tensor=bp.tensor, offset=bp.offset, ap=[[0, 128], [1, C]])
    nc.sync.dma_start(bp_sb[:], bp_bcast)

    # ---------- phase 2: causal attention, S^T layout, heads interleaved ----------
    # Interleaving the two heads keeps consecutive PE matmuls independent
    # (different array row groups for S, different PSUM banks throughout),
    # so LDWEIGHTS/fill/drain overlap instead of serializing.
    ph23 = ExitStack()
    spool = ph23.enter_context(tc.tile_pool(name="sps", bufs=2, space="PSUM"))
    ypool = ph23.enter_context(tc.tile_pool(name="yps", bufs=2, space="PSUM"))
    ptpool = ph23.enter_context(tc.tile_pool(name="pt", bufs=3))
    npool = ph23.enter_context(tc.tile_pool(name="norm", bufs=3))
    yT = (yT0, yT1)
    for b in range(B):
        for qb in range(NQB):
            q0g = b * T + qb * QB
            njt = (qb + 1) * (QB // 128)
            yps = [
                ypool.tile([HD + 1, QB], F32, tag="yps", name=f"yp{b}_{qb}_{h}")
                for h in range(HPC)
            ]
            for j in range(njt):
                j0g = b * T + j * 128
                diag = j * 128 + 127 > qb * QB
                i0 = max(0, j * 128 - qb * QB)  # first unmasked query col
                pts = []
                for h in range(HPC):
                    hs = slice(h * HD, (h + 1) * HD)
                    sp = spool.tile([128, QB], F32, tag=f"sps{h}")
                    nc.tensor.matmul(
                        sp[:, i0:QB], lhsT=kT[hs, j0g : j0g + 128],
                        rhs=qT[hs, q0g + i0 : q0g + QB], start=True, stop=True,
                    )
                    pt = ptpool.tile([128, QB], BF16, tag=f"pt{h}")
                    if i0:
                        nc.vector.memset(pt[:, 0:i0], 0.0)
                    nc.scalar.activation(
                        pt[:, i0:QB], sp[:, i0:QB], AF.Exp, scale=float(SCALE)
                    )
                    if diag:
                        # boundary tile: keep where j <= i on the 128-wide band
                        nc.gpsimd.affine_select(
                            pt[:, i0 : i0 + 128], pt[:, i0 : i0 + 128],
                            pattern=[[1, 128]], base=0, channel_multiplier=-1,
                            compare_op=mybir.AluOpType.is_ge, fill=0.0,
                        )
                    pts.append(pt)
                for h in range(HPC):
                    nc.tensor.matmul(
                        yps[h][:], lhsT=vsb[:, b * JTN + j, h, 0 : HD + 1],
                        rhs=pts[h][:], start=(j == 0), stop=(j == njt - 1),
                    )
            # softmax normalization: row HD of yp is the denominator.
            # One fast PSUM->SBUF copy releases the yp bank; the recip /
            # broadcast / scale chain then runs off SBUF.
            for h in range(HPC):
                ln = npool.tile([1, QB], F32, tag="ln")
                nc.vector.tensor_copy(ln[:], yps[h][HD : HD + 1, :])
                yraw = npool.tile([HD, QB], F32, tag="yraw")
                nc.vector.tensor_copy(yraw[:], yps[h][0:HD, :])
                rn = npool.tile([1, QB], F32, tag="rn")
                sc = npool.tile([1, QB], F32, tag="sc")
                nc.vector.reciprocal_approx_accurate(rn[:], ln[:], sc[:])
                rb = npool.tile([HD, QB], F32, tag="rb")
                nc.gpsimd.partition_broadcast(rb[:], rn[:], channels=HD)
                nc.vector.tensor_mul(yT[h][:, q0g : q0g + QB], yraw[:], rb[:])

    ph23.close()

    # ---------- phase 3: single AllToAll over rows, then output projection.
    # Kept strictly after all compute: collectives running concurrently with
    # compute-era DMA traffic corrupted data nondeterministically on HW.
    for r in range(N_CORES):
        nc.sync.dma_start(cc_in[r, 0:HD, :], yT0[:, r * ROWS : (r + 1) * ROWS])
        nc.sync.dma_start(cc_in[r, HD:FPC, :], yT1[:, r * ROWS : (r + 1) * ROWS])
    cc = nc.gpsimd.collective_compute(
        "AllToAll", mybir.AluOpType.bypass,
        ins=[cc_in[:]], outs=[cc_out[:]],
        replica_groups=[list(range(N_CORES))],
    )
    for ti in transpose_insts:
        add_dep_helper(cc.ins, ti.ins, True, "serialize transposes before A2A")
    yfull = res.tile([128, KC, ROWS], BF16)
    nc.sync.dma_start(yfull[:], cc_out.rearrange("r p t -> p r t"))
    opool = ctx.enter_context(tc.tile_pool(name="ops", bufs=2, space="PSUM"))
    ospool = ctx.enter_context(tc.tile_pool(name="osb", bufs=3))
    for tt in range(ROWS // 128):
        ps0 = opool.tile([128, 512], F32, tag="ops0")
        ps1 = opool.tile([128, 512], F32, tag="ops1")
        for a in range(KC):
            lhsT = yfull[:, a, tt * 128 : (tt + 1) * 128]
            nc.tensor.matmul(ps0[:], lhsT=lhsT, rhs=wp_sb[:, a, 0:512],
                             start=(a == 0), stop=(a == KC - 1))
            nc.tensor.matmul(ps1[:], lhsT=lhsT, rhs=wp_sb[:, a, 512:C],
                             start=(a == 0), stop=(a == KC - 1))
        osb = ospool.tile([128, C], F32, tag="osb")
        nc.vector.tensor_add(osb[:, 0:512], ps0[:], bp_sb[:, 0:512])
        nc.vector.tensor_add(osb[:, 512:C], ps1[:], bp_sb[:, 512:C])
        nc.sync.dma_start(out[tt * 128 : (tt + 1) * 128, :], osb[:])

    if dbg is not None:
        nc.sync.dma_start(dbg["d_qT"][:], qT[:])
        nc.sync.dma_start(dbg["d_kT"][:], kT[:])
        nc.sync.dma_start(dbg["d_vT"][:], vT_dram[:])
        nc.sync.dma_start(dbg["d_vsb"][:, :, :, 0 : HD + 1], vsb[:, :, :, 0 : HD + 1])
        nc.sync.dma_start(dbg["d_yT0"][:], yT0[:])
        nc.sync.dma_start(dbg["d_yT1"][:], yT1[:])


_COMPILED_NC = None


def _get_nc():
    global _COMPILED_NC
    if _COMPILED_NC is None:
        nc = bacc.Bacc("TRN2", target_bir_lowering=False, debug=False,
                       num_devices=N_CORES)
        build_program(nc)
        nc.compile()
        _COMPILED_NC = nc
    return _COMPILED_NC


def kernel(x, W_attn, b_attn, W_proj, b_proj):
    global LAST_RESULTS
    nc = _get_nc()

    bf = ml_dtypes.bfloat16
    xT_np = np.ascontiguousarray(
        np.asarray(x, np.float32).reshape(BT, C).T
    ).astype(bf)
    W_attn = np.asarray(W_attn, np.float32)
    b_attn = np.asarray(b_attn, np.float32)
    wp_np = np.asarray(W_proj, np.float32).astype(bf)
    bp_np = np.asarray(b_proj, np.float32)

    in_maps = []
    for c in range(N_CORES):
        s = slice(c * FPC, (c + 1) * FPC)
        in_maps.append({
            "xT": xT_np,
            "wq": np.ascontiguousarray(W_attn[:, s]).astype(bf),
            "wk": np.ascontiguousarray(W_attn[:, C:2 * C][:, s]).astype(bf),
            "wv": np.ascontiguousarray(W_attn[:, 2 * C:][:, s]).astype(bf),
            "bqkv": np.ascontiguousarray(
                np.stack([b_attn[s], b_attn[C:2 * C][s], b_attn[2 * C:][s]])
            ).astype(np.float32),
            "wp": wp_np,
            "bp": bp_np,
        })

    res = run_bass_kernel_spmd(nc, in_maps, core_ids=list(range(N_CORES)))
    LAST_RESULTS = res
    full = np.concatenate([res.results[c]["out"] for c in range(N_CORES)], axis=0)
    return full.reshape(B, T, C)



# revision 2
# speedup vs baseline: 1.6230x; 1.6230x over previous
"""Causal self-attention (GPT-style block) on 8 Trainium2 NeuronCores.

Problem: x[4,2048,1024] -> qkv = x@W_attn+b ; 16-head causal attention
(head_dim 64) ; out = y@W_proj+b_proj.

Sharding: tensor-parallel over heads. Core c owns heads {2c, 2c+1}:
  - computes q^T/k^T for its heads over the full batch via matmuls against
    a host-pretransposed x^T (bf16); v is computed directly in NATURAL
    layout (rows on partitions) by swapping matmul roles (x^T tile
    stationary, W_v moving) — no DRAM roundtrip / DMA transposes,
  - runs causal attention for its 8 (batch, head) pairs entirely in SBUF
    (S^T layout: scores tile [128 j, 512 i]; both heads' scores share one
    2-bank PSUM tile so a single wide exp on ScalarE covers them; causal
    mask on GpSimd; PV matmul with a ones-column appended to V producing
    both y_raw^T and the softmax denominator in one PSUM tile),
  - qkv compute for batch b+1 is interleaved into the attention blocks of
    batch b so the PE queue never drains (PE p-state ramp: idle resets
    the clock to 0.65GHz, 3us of continuous busy to reach 2.4GHz),
  - S(j) is emitted one tile ahead of PV(j-1) so PV never heads the PE
    queue while its exp is still in flight,
  - AllToAll inputs are staged with plain DMAs as each query block
    normalizes; one AllToAll at the end swaps the head dim for the row
    dim, after which each core computes the output projection for its
    1024 rows, pipelined against the per-row-tile gather DMA.

Numerics: bf16 operands with fp32 PSUM accumulation everywhere; softmax
skips the max-subtraction (scores are O(1) by construction; exp stays
finite) which matches the reference to ~5e-3 in fp32.
"""

import numpy as np
import ml_dtypes
from contextlib import ExitStack

import concourse.bass as bass
import concourse.tile as tile
from concourse import bacc, mybir
from concourse.tile_rust import add_dep_helper
from concourse.bass_utils import run_bass_kernel_spmd

F32 = mybir.dt.float32
BF16 = mybir.dt.bfloat16
AF = mybir.ActivationFunctionType

N_CORES = 8
B, T, C, H = 4, 2048, 1024, 16
HD = C // H            # 64 head dim
HPC = H // N_CORES     # 2 heads per core
FPC = HPC * HD         # 128 features per core
BT = B * T             # 8192 rows
TCHUNK = 512           # t chunk in qkv phase
NT_CHUNKS = BT // TCHUNK
QB = 512               # query block
NQB = T // QB          # 4 per batch
JTN = T // 128         # 16 j-tiles per batch
ROWS = BT // N_CORES   # 1024 rows per core after AllToAll
KC = C // 128          # 8 contraction tiles over C
VW = 80                # [V | 1 | pad] row unit
SCALE = 1.0 / np.sqrt(HD)

LAST_RESULTS = None    # test.py reads exec_time_ns off this


def build_program(nc):
    xT = nc.dram_tensor("xT", [C, BT], BF16, kind="ExternalInput").ap()
    wq = nc.dram_tensor("wq", [C, FPC], BF16, kind="ExternalInput").ap()
    wk = nc.dram_tensor("wk", [C, FPC], BF16, kind="ExternalInput").ap()
    wv = nc.dram_tensor("wv", [C, FPC], BF16, kind="ExternalInput").ap()
    bqkv = nc.dram_tensor("bqkv", [3, FPC], F32, kind="ExternalInput").ap()
    wp = nc.dram_tensor("wp", [C, C], BF16, kind="ExternalInput").ap()
    bp = nc.dram_tensor("bp", [C], F32, kind="ExternalInput").ap()
    out = nc.dram_tensor("out", [ROWS, C], F32, kind="ExternalOutput").ap()
    cc_in = nc.dram_tensor("cc_in", [N_CORES, FPC, ROWS], BF16, kind="Internal").ap()
    cc_out = nc.dram_tensor("cc_out", [N_CORES, FPC, ROWS], BF16, kind="Internal").ap()

    with tile.TileContext(nc) as tc:
        with ExitStack() as ctx:
            emit(ctx, tc, xT, wq, wk, wv, bqkv, wp, bp, out, cc_in, cc_out)
    return nc


def emit(ctx, tc, xT, wq, wk, wv, bqkv, wp, bp, out, cc_in, cc_out):
    nc = tc.nc
    res = ctx.enter_context(tc.tile_pool(name="resident", bufs=1))

    # ---------- resident SBUF ----------
    qT = res.tile([128, BT], BF16)
    kT = res.tile([128, BT], BF16)
    vsb = res.tile([128, B * JTN, HPC, VW], BF16)     # [V | 1 | pad] per j-tile/head
    wq_sb = res.tile([128, KC, FPC], BF16)
    wk_sb = res.tile([128, KC, FPC], BF16)
    wv_sb = res.tile([128, KC, FPC], BF16)
    b_sb = res.tile([128, 3], F32)
    bv_sb = res.tile([128, 4, HPC, HD], F32)          # v-bias, free-axis broadcast
    wp_sb = res.tile([128, KC, C], BF16)
    bp_sb = res.tile([128, C], F32)
    yT0 = res.tile([64, BT], BF16)
    yT1 = res.tile([64, BT], BF16)
    yT = (yT0, yT1)

    # ---------- constant/weight loads ----------
    nc.sync.dma_start(wq_sb[:], wq.rearrange("(a p) m -> p a m", p=128))
    nc.sync.dma_start(wk_sb[:], wk.rearrange("(a p) m -> p a m", p=128))
    nc.sync.dma_start(wv_sb[:], wv.rearrange("(a p) m -> p a m", p=128))
    nc.sync.dma_start(b_sb[:], bqkv.rearrange("b p -> p b"))
    bv_bcast = bass.AP(
        tensor=bqkv.tensor, offset=bqkv.offset + 2 * FPC,
        ap=[[0, 128], [0, 4], [HD, HPC], [1, HD]],
    )
    nc.sync.dma_start(bv_sb[:], bv_bcast)
    nc.sync.dma_start(wp_sb[:], wp.rearrange("(a p) e -> p a e", p=128))
    bp_bcast = bass.AP(tensor=bp.tensor, offset=bp.offset, ap=[[0, 128], [1, C]])
    nc.sync.dma_start(bp_sb[:], bp_bcast)
    nc.vector.memset(vsb[:, :, :, HD : HD + 1], 1.0)

    ph = ExitStack()
    xpool = ph.enter_context(tc.tile_pool(name="xt", bufs=3))
    gpool = ph.enter_context(tc.tile_pool(name="gps", bufs=2, space="PSUM"))
    spool = ph.enter_context(tc.tile_pool(name="sps", bufs=2, space="PSUM"))
    ypool = ph.enter_context(tc.tile_pool(name="yps", bufs=2, space="PSUM"))
    ptpool = ph.enter_context(tc.tile_pool(name="pt", bufs=3))
    npool = ph.enter_context(tc.tile_pool(name="norm", bufs=3))

    xT_t = xT.rearrange("(a p) t -> p a t", p=128)
    staging = []

    def emit_qkv_chunk(tci):
        """q^T/k^T (weights stationary) + natural-layout v (x^T stationary)
        for one 512-row chunk."""
        t0 = tci * TCHUNK
        xt = xpool.tile([128, KC, TCHUNK], BF16, tag="xt", name=f"xt{tci}")
        for spl in range(4):
            nc.sync.dma_start(
                xt[:, 2 * spl : 2 * spl + 2, :],
                xT_t[:, 2 * spl : 2 * spl + 2, t0 : t0 + TCHUNK],
            )
        for w_sb, bi, dst in ((wq_sb, 0, qT), (wk_sb, 1, kT)):
            ps = gpool.tile([128, TCHUNK], F32, tag="g", name=f"ps{tci}_{bi}")
            for a in range(KC):
                nc.tensor.matmul(
                    ps[:], lhsT=w_sb[:, a, :], rhs=xt[:, a, :],
                    start=(a == 0), stop=(a == KC - 1),
                )
            nc.vector.tensor_scalar_add(
                dst[:, t0 : t0 + TCHUNK], ps[:], b_sb[:, bi : bi + 1]
            )
        vp = gpool.tile([128, 4, HPC, HD], F32, tag="g", name=f"vp{tci}")
        for sub in range(4):
            for a in range(KC):
                nc.tensor.matmul(
                    vp[:, sub], lhsT=xt[:, a, sub * 128 : (sub + 1) * 128],
                    rhs=wv_sb[:, a, :], start=(a == 0), stop=(a == KC - 1),
                )
        g0 = tci * 4
        nc.vector.tensor_add(vsb[:, g0 : g0 + 4, :, 0:HD], vp[:], bv_sb[:])

    def emit_pv(b, j, pt, yps, njt):
        for h in range(HPC):
            nc.tensor.matmul(
                yps[h][:], lhsT=vsb[:, b * JTN + j, h, 0 : HD + 1],
                rhs=pt[:, h, :], start=(j == 0), stop=(j == njt - 1),
            )

    def emit_attn_block(b, qb):
        """Causal attention for one (batch, query-block): S^T for both heads
        into a 2-bank PSUM pair, one wide exp, PV one j behind S."""
        q0g = b * T + qb * QB
        njt = (qb + 1) * (QB // 128)
        yps = [
            ypool.tile([HD + 1, QB], F32, tag="yp", name=f"yp{b}_{qb}_{h}")
            for h in range(HPC)
        ]
        pts = []
        for j in range(njt):
            j0g = b * T + j * 128
            diag = j * 128 + 127 > qb * QB
            i0 = max(0, j * 128 - qb * QB)
            sp = spool.tile([128, HPC, QB], F32, tag="sp", name=f"sp{b}_{qb}_{j}")
            for h in range(HPC):
                hs = slice(h * HD, (h + 1) * HD)
                nc.tensor.matmul(
                    sp[:, h, i0:QB], lhsT=kT[hs, j0g : j0g + 128],
                    rhs=qT[hs, q0g + i0 : q0g + QB], start=True, stop=True,
                )
            pt = ptpool.tile([128, HPC, QB], BF16, tag="pt", name=f"pt{b}_{qb}_{j}")
            if i0:
                nc.vector.memset(pt[:, :, 0:i0], 0.0)
            nc.scalar.activation(
                pt[:, :, i0:QB], sp[:, :, i0:QB], AF.Exp, scale=float(SCALE)
            )
            if diag:
                for h in range(HPC):
                    nc.gpsimd.affine_select(
                        pt[:, h, i0 : i0 + 128], pt[:, h, i0 : i0 + 128],
                        pattern=[[1, 128]], base=0, channel_multiplier=-1,
                        compare_op=mybir.AluOpType.is_ge, fill=0.0,
                    )
            pts.append(pt)
            if j > 0:
                emit_pv(b, j - 1, pts[j - 1], yps, njt)
        emit_pv(b, njt - 1, pts[njt - 1], yps, njt)

        # softmax normalization: row HD of yp is the denominator; then stage
        # this query block's rows for the AllToAll (plain DMA, overlapped).
        r = (b * T + qb * QB) // ROWS
        col0 = (b * T + qb * QB) % ROWS
        for h in range(HPC):
            ln = npool.tile([1, QB], F32, tag="ln")
            nc.vector.tensor_copy(ln[:], yps[h][HD : HD + 1, :])
            yraw = npool.tile([HD, QB], F32, tag="yraw")
            nc.vector.tensor_copy(yraw[:], yps[h][0:HD, :])
            rn = npool.tile([1, QB], F32, tag="rn")
            sc = npool.tile([1, QB], F32, tag="sc")
            nc.vector.reciprocal_approx_accurate(rn[:], ln[:], sc[:])
            rb = npool.tile([HD, QB], F32, tag="rb")
            nc.gpsimd.partition_broadcast(rb[:], rn[:], channels=HD)
            nc.vector.tensor_mul(yT[h][:, q0g : q0g + QB], yraw[:], rb[:])
            ti = nc.sync.dma_start(
                cc_in[r, h * HD : (h + 1) * HD, col0 : col0 + QB],
                yT[h][:, q0g : q0g + QB],
            )
            staging.append(ti)

    # ---------- fused qkv + attention pipeline ----------
    for tci in range(NQB):
        emit_qkv_chunk(tci)
    for b in range(B):
        for qb in range(NQB):
            emit_attn_block(b, qb)
            if b + 1 < B:
                emit_qkv_chunk((b + 1) * NQB + qb)

    ph.close()

    # ---------- AllToAll over rows, then output projection ----------
    cc = nc.gpsimd.collective_compute(
        "AllToAll", mybir.AluOpType.bypass,
        ins=[cc_in[:]], outs=[cc_out[:]],
        replica_groups=[list(range(N_CORES))],
    )
    for ti in staging:
        add_dep_helper(cc.ins, ti.ins, True, "staging DMAs before A2A")

    opool = ctx.enter_context(tc.tile_pool(name="ops", bufs=2, space="PSUM"))
    ospool = ctx.enter_context(tc.tile_pool(name="osb", bufs=3))
    yfpool = ctx.enter_context(tc.tile_pool(name="yf", bufs=3))
    for tt in range(ROWS // 128):
        yf = yfpool.tile([128, KC, 128], BF16, tag="yf", name=f"yf{tt}")
        src = bass.AP(
            tensor=cc_out.tensor, offset=cc_out.offset + tt * 128,
            ap=[[ROWS, 128], [FPC * ROWS, N_CORES], [1, 128]],
        )
        nc.sync.dma_start(yf[:], src)
        ps0 = opool.tile([128, 512], F32, tag="o0", name=f"o0_{tt}")
        ps1 = opool.tile([128, 512], F32, tag="o1", name=f"o1_{tt}")
        for a in range(KC):
            nc.tensor.matmul(ps0[:], lhsT=yf[:, a, :], rhs=wp_sb[:, a, 0:512],
                             start=(a == 0), stop=(a == KC - 1))
            nc.tensor.matmul(ps1[:], lhsT=yf[:, a, :], rhs=wp_sb[:, a, 512:C],
                             start=(a == 0), stop=(a == KC - 1))
        osb = ospool.tile([128, C], F32, tag="osb", name=f"osb{tt}")
        nc.vector.tensor_add(osb[:, 0:512], ps0[:], bp_sb[:, 0:512])
        nc.vector.tensor_add(osb[:, 512:C], ps1[:], bp_sb[:, 512:C])
        nc.sync.dma_start(out[tt * 128 : (tt + 1) * 128, :], osb[:])


_COMPILED_NC = None


def _get_nc():
    global _COMPILED_NC
    if _COMPILED_NC is None:
        nc = bacc.Bacc("TRN2", target_bir_lowering=False, debug=False,
                       num_devices=N_CORES)
        build_program(nc)
        nc.compile()
        _COMPILED_NC = nc
    return _COMPILED_NC


def kernel(x, W_attn, b_attn, W_proj, b_proj):
    global LAST_RESULTS
    nc = _get_nc()

    bf = ml_dtypes.bfloat16
    xT_np = np.ascontiguousarray(
        np.asarray(x, np.float32).reshape(BT, C).T
    ).astype(bf)
    W_attn = np.asarray(W_attn, np.float32)
    b_attn = np.asarray(b_attn, np.float32)
    wp_np = np.asarray(W_proj, np.float32).astype(bf)
    bp_np = np.asarray(b_proj, np.float32)

    in_maps = []
    for c in range(N_CORES):
        s = slice(c * FPC, (c + 1) * FPC)
        in_maps.append({
            "xT": xT_np,
            "wq": np.ascontiguousarray(W_attn[:, s]).astype(bf),
            "wk": np.ascontiguousarray(W_attn[:, C:2 * C][:, s]).astype(bf),
            "wv": np.ascontiguousarray(W_attn[:, 2 * C:][:, s]).astype(bf),
            "bqkv": np.ascontiguousarray(
                np.stack([b_attn[s], b_attn[C:2 * C][s], b_attn[2 * C:][s]])
            ).astype(np.float32),
            "wp": wp_np,
            "bp": bp_np,
        })

    res = run_bass_kernel_spmd(nc, in_maps, core_ids=list(range(N_CORES)))
    LAST_RESULTS = res
    full = np.concatenate([res.results[c]["out"] for c in range(N_CORES)], axis=0)
    return full.reshape(B, T, C)


# revision 4
# speedup vs baseline: 1.8249x; 1.1244x over previous
"""Causal self-attention (GPT-style block) on 8 Trainium2 NeuronCores.

Problem: x[4,2048,1024] -> qkv = x@W_attn+b ; 16-head causal attention
(head_dim 64) ; out = y@W_proj+b_proj.

Sharding: tensor-parallel over heads. Core c owns heads {2c, 2c+1}:
  - computes q^T/k^T for its heads over the full batch via matmuls against
    a host-pretransposed x^T (bf16); v is computed directly in NATURAL
    layout (rows on partitions) by swapping matmul roles (x^T tile
    stationary, W_v moving) — no DRAM roundtrip / DMA transposes,
  - runs causal attention for its 8 (batch, head) pairs entirely in SBUF
    (S^T layout: scores tile [128 j, 512 i]; both heads' scores share one
    2-bank PSUM tile so a single wide exp on ScalarE covers them; causal
    mask on GpSimd; PV matmul with a ones-column appended to V producing
    both y_raw^T and the softmax denominator in one PSUM tile),
  - qkv compute for batch b+1 is interleaved into the attention blocks of
    batch b so the PE queue never drains (PE p-state ramp: idle resets
    the clock to 0.65GHz, 3us of continuous busy to reach 2.4GHz),
  - S(j) is emitted one tile ahead of PV(j-1) so PV never heads the PE
    queue while its exp is still in flight,
  - the head->row AllToAll is split in two: core r owns 512-row blocks
    (b=r//4, qb=r%4) and (b=2+r//4, qb=r%4).  The batches-0/1 exchange
    fires as soon as attn(1) normalizes and runs under attn(2/3) compute;
    its output projection tiles interleave into attn(3)'s blocks.  Only
    the batches-2/3 exchange + 4 projection tiles remain as tail.
    The host reorders the 16 row blocks back to batch-major order.

Numerics: bf16 operands with fp32 PSUM accumulation everywhere; softmax
skips the max-subtraction (scores are O(1) by construction; exp stays
finite) which matches the reference to ~5e-3 in fp32.
"""

import numpy as np
import ml_dtypes
from contextlib import ExitStack

import concourse.bass as bass
import concourse.tile as tile
from concourse import bacc, mybir
from concourse.tile_rust import add_dep_helper
from concourse.bass_utils import run_bass_kernel_spmd

F32 = mybir.dt.float32
BF16 = mybir.dt.bfloat16
AF = mybir.ActivationFunctionType

N_CORES = 8
B, T, C, H = 4, 2048, 1024, 16
HD = C // H            # 64 head dim
HPC = H // N_CORES     # 2 heads per core
FPC = HPC * HD         # 128 features per core
BT = B * T             # 8192 rows
TCHUNK = 512           # t chunk in qkv phase
NT_CHUNKS = BT // TCHUNK
QB = 512               # query block
NQB = T // QB          # 4 per batch
JTN = T // 128         # 16 j-tiles per batch
ROWS = BT // N_CORES   # 1024 rows per core after the exchanges
KC = C // 128          # 8 contraction tiles over C
VW = 80                # [V | 1 | pad] row unit
SCALE = 1.0 / np.sqrt(HD)

LAST_RESULTS = None    # test.py reads exec_time_ns off this


def build_program(nc):
    xT = nc.dram_tensor("xT", [C, BT], BF16, kind="ExternalInput").ap()
    wq = nc.dram_tensor("wq", [C, FPC], BF16, kind="ExternalInput").ap()
    wk = nc.dram_tensor("wk", [C, FPC], BF16, kind="ExternalInput").ap()
    wv = nc.dram_tensor("wv", [C, FPC], BF16, kind="ExternalInput").ap()
    bqkv = nc.dram_tensor("bqkv", [3, FPC], F32, kind="ExternalInput").ap()
    wp = nc.dram_tensor("wp", [C, C], BF16, kind="ExternalInput").ap()
    bp = nc.dram_tensor("bp", [C], F32, kind="ExternalInput").ap()
    out = nc.dram_tensor("out", [ROWS, C], F32, kind="ExternalOutput").ap()
    cc_inA = nc.dram_tensor("cc_inA", [N_CORES, FPC, QB], BF16, kind="Internal").ap()
    cc_outA = nc.dram_tensor("cc_outA", [N_CORES, FPC, QB], BF16, kind="Internal").ap()
    cc_inB = nc.dram_tensor("cc_inB", [N_CORES, FPC, QB], BF16, kind="Internal").ap()
    cc_outB = nc.dram_tensor("cc_outB", [N_CORES, FPC, QB], BF16, kind="Internal").ap()

    with tile.TileContext(nc) as tc:
        with ExitStack() as ctx:
            emit(ctx, tc, xT, wq, wk, wv, bqkv, wp, bp, out,
                 (cc_inA, cc_outA), (cc_inB, cc_outB))
    return nc


def emit(ctx, tc, xT, wq, wk, wv, bqkv, wp, bp, out, ccA, ccB):
    nc = tc.nc
    cc_inA, cc_outA = ccA
    cc_inB, cc_outB = ccB
    res = ctx.enter_context(tc.tile_pool(name="resident", bufs=1))

    # ---------- resident SBUF ----------
    qT = res.tile([128, BT], BF16)
    kT = res.tile([128, BT], BF16)
    vsb = res.tile([128, B * JTN, HPC, VW], BF16)     # [V | 1 | pad] per j-tile/head
    wq_sb = res.tile([128, KC, FPC], BF16)
    wk_sb = res.tile([128, KC, FPC], BF16)
    wv_sb = res.tile([128, KC, FPC], BF16)
    b_sb = res.tile([128, 3], F32)
    bv_sb = res.tile([128, 4, HPC, HD], F32)          # v-bias, free-axis broadcast
    wp_sb = res.tile([128, KC, C], BF16)
    bp_sb = res.tile([128, C], F32)
    yT0 = res.tile([64, BT], BF16)
    yT1 = res.tile([64, BT], BF16)
    yT = (yT0, yT1)

    # ---------- constant/weight loads ----------
    nc.sync.dma_start(wq_sb[:], wq.rearrange("(a p) m -> p a m", p=128))
    nc.sync.dma_start(wk_sb[:], wk.rearrange("(a p) m -> p a m", p=128))
    nc.sync.dma_start(wv_sb[:], wv.rearrange("(a p) m -> p a m", p=128))
    nc.sync.dma_start(b_sb[:], bqkv.rearrange("b p -> p b"))
    bv_bcast = bass.AP(
        tensor=bqkv.tensor, offset=bqkv.offset + 2 * FPC,
        ap=[[0, 128], [0, 4], [HD, HPC], [1, HD]],
    )
    nc.sync.dma_start(bv_sb[:], bv_bcast)
    nc.sync.dma_start(wp_sb[:], wp.rearrange("(a p) e -> p a e", p=128))
    bp_bcast = bass.AP(tensor=bp.tensor, offset=bp.offset, ap=[[0, 128], [1, C]])
    nc.sync.dma_start(bp_sb[:], bp_bcast)
    nc.vector.memset(vsb[:, :, :, HD : HD + 1], 1.0)

    ospool = ctx.enter_context(tc.tile_pool(name="osb", bufs=3))
    yfpool = ctx.enter_context(tc.tile_pool(name="yf", bufs=3))
    ph = ExitStack()
    xpool = ph.enter_context(tc.tile_pool(name="xt", bufs=3))
    gpool = ph.enter_context(tc.tile_pool(name="gps", bufs=2, space="PSUM"))
    spool = ph.enter_context(tc.tile_pool(name="sps", bufs=2, space="PSUM"))
    ypool = ph.enter_context(tc.tile_pool(name="yps", bufs=2, space="PSUM"))
    ptpool = ph.enter_context(tc.tile_pool(name="pt", bufs=3))
    npool = ph.enter_context(tc.tile_pool(name="norm", bufs=3))

    xT_t = xT.rearrange("(a p) t -> p a t", p=128)
    stagingA, stagingB = [], []

    def emit_qkv_chunk(tci):
        """q^T/k^T (weights stationary) + natural-layout v (x^T stationary)
        for one 512-row chunk."""
        t0 = tci * TCHUNK
        xt = xpool.tile([128, KC, TCHUNK], BF16, tag="xt", name=f"xt{tci}")
        for spl in range(4):
            nc.sync.dma_start(
                xt[:, 2 * spl : 2 * spl + 2, :],
                xT_t[:, 2 * spl : 2 * spl + 2, t0 : t0 + TCHUNK],
            )
        for w_sb, bi, dst in ((wq_sb, 0, qT), (wk_sb, 1, kT)):
            ps = gpool.tile([128, TCHUNK], F32, tag="g", name=f"ps{tci}_{bi}")
            for a in range(KC):
                nc.tensor.matmul(
                    ps[:], lhsT=w_sb[:, a, :], rhs=xt[:, a, :],
                    start=(a == 0), stop=(a == KC - 1),
                )
            nc.vector.tensor_scalar_add(
                dst[:, t0 : t0 + TCHUNK], ps[:], b_sb[:, bi : bi + 1]
            )
        vp = gpool.tile([128, 4, HPC, HD], F32, tag="g", name=f"vp{tci}")
        for sub in range(4):
            for a in range(KC):
                nc.tensor.matmul(
                    vp[:, sub], lhsT=xt[:, a, sub * 128 : (sub + 1) * 128],
                    rhs=wv_sb[:, a, :], start=(a == 0), stop=(a == KC - 1),
                )
        g0 = tci * 4
        nc.vector.tensor_add(vsb[:, g0 : g0 + 4, :, 0:HD], vp[:], bv_sb[:])

    def emit_pv(b, j, pt, yps, njt):
        for h in range(HPC):
            nc.tensor.matmul(
                yps[h][:], lhsT=vsb[:, b * JTN + j, h, 0 : HD + 1],
                rhs=pt[:, h, :], start=(j == 0), stop=(j == njt - 1),
            )

    def emit_attn_block(b, qb):
        """Causal attention for one (batch, query-block): S^T for both heads
        into a 2-bank PSUM pair, one wide exp, PV one j behind S."""
        q0g = b * T + qb * QB
        njt = (qb + 1) * (QB // 128)
        yps = [
            ypool.tile([HD + 1, QB], F32, tag="yp", name=f"yp{b}_{qb}_{h}")
            for h in range(HPC)
        ]
        pts = []
        for j in range(njt):
            j0g = b * T + j * 128
            diag = j * 128 + 127 > qb * QB
            i0 = max(0, j * 128 - qb * QB)
            sp = spool.tile([128, HPC, QB], F32, tag="sp", name=f"sp{b}_{qb}_{j}")
            for h in range(HPC):
                hs = slice(h * HD, (h + 1) * HD)
                nc.tensor.matmul(
                    sp[:, h, i0:QB], lhsT=kT[hs, j0g : j0g + 128],
                    rhs=qT[hs, q0g + i0 : q0g + QB], start=True, stop=True,
                )
            pt = ptpool.tile([128, HPC, QB], BF16, tag="pt", name=f"pt{b}_{qb}_{j}")
            if i0:
                nc.vector.memset(pt[:, :, 0:i0], 0.0)
            nc.scalar.activation(
                pt[:, :, i0:QB], sp[:, :, i0:QB], AF.Exp, scale=float(SCALE)
            )
            if diag:
                for h in range(HPC):
                    nc.gpsimd.affine_select(
                        pt[:, h, i0 : i0 + 128], pt[:, h, i0 : i0 + 128],
                        pattern=[[1, 128]], base=0, channel_multiplier=-1,
                        compare_op=mybir.AluOpType.is_ge, fill=0.0,
                    )
            pts.append(pt)
            if j > 0:
                emit_pv(b, j - 1, pts[j - 1], yps, njt)
        emit_pv(b, njt - 1, pts[njt - 1], yps, njt)

        # softmax normalization: row HD of yp is the denominator; then stage
        # this 512-row block for its exchange (plain DMA, overlapped).
        cc_in = cc_inA if b < 2 else cc_inB
        staging = stagingA if b < 2 else stagingB
        r = (b % 2) * 4 + qb
        for h in range(HPC):
            ln = npool.tile([1, QB], F32, tag="ln")
            nc.vector.tensor_copy(ln[:], yps[h][HD : HD + 1, :])
            yraw = npool.tile([HD, QB], F32, tag="yraw")
            nc.vector.tensor_copy(yraw[:], yps[h][0:HD, :])
            rn = npool.tile([1, QB], F32, tag="rn")
            sc = npool.tile([1, QB], F32, tag="sc")
            nc.vector.reciprocal_approx_accurate(rn[:], ln[:], sc[:])
            rb = npool.tile([HD, QB], F32, tag="rb")
            nc.gpsimd.partition_broadcast(rb[:], rn[:], channels=HD)
            nc.vector.tensor_mul(yT[h][:, q0g : q0g + QB], yraw[:], rb[:])
            ti = nc.sync.dma_start(
                cc_in[r, h * HD : (h + 1) * HD, :],
                yT[h][:, q0g : q0g + QB],
            )
            staging.append(ti)

    def emit_proj_tile(cc_out, tt, row0, psum_pool, ptag):
        """One 128-row output-projection tile from an exchanged block."""
        yf = yfpool.tile([128, KC, 128], BF16, tag="yf", name=f"yf{row0}_{tt}")
        src = bass.AP(
            tensor=cc_out.tensor, offset=cc_out.offset + tt * 128,
            ap=[[QB, 128], [FPC * QB, N_CORES], [1, 128]],
        )
        nc.sync.dma_start(yf[:], src)
        ps0 = psum_pool.tile([128, 512], F32, tag=ptag, name=f"op0_{row0}_{tt}")
        ps1 = psum_pool.tile([128, 512], F32, tag=ptag, name=f"op1_{row0}_{tt}")
        for a in range(KC):
            nc.tensor.matmul(ps0[:], lhsT=yf[:, a, :], rhs=wp_sb[:, a, 0:512],
                             start=(a == 0), stop=(a == KC - 1))
            nc.tensor.matmul(ps1[:], lhsT=yf[:, a, :], rhs=wp_sb[:, a, 512:C],
                             start=(a == 0), stop=(a == KC - 1))
        osb = ospool.tile([128, C], F32, tag="osb", name=f"osb{row0}_{tt}")
        nc.vector.tensor_add(osb[:, 0:512], ps0[:], bp_sb[:, 0:512])
        nc.vector.tensor_add(osb[:, 512:C], ps1[:], bp_sb[:, 512:C])
        r0 = row0 + tt * 128
        nc.sync.dma_start(out[r0 : r0 + 128, :], osb[:])

    # ---------- fused qkv + attention pipeline ----------
    for tci in range(NQB):
        emit_qkv_chunk(tci)
    ccA_inst = None
    for b in range(B):
        for qb in range(NQB):
            emit_attn_block(b, qb)
            if b + 1 < B:
                emit_qkv_chunk((b + 1) * NQB + qb)
            if b == 3:
                # blockA projection rides inside attn(3): its exchange
                # finished during attn(2).
                emit_proj_tile(cc_outA, qb, 0, gpool, "g")
        if b == 1:
            # batches-0/1 exchange: runs under attn(2/3) compute.
            ccA_inst = nc.gpsimd.collective_compute(
                "AllToAll", mybir.AluOpType.bypass,
                ins=[cc_inA[:]], outs=[cc_outA[:]],
                replica_groups=[list(range(N_CORES))],
            )
            for ti in stagingA:
                add_dep_helper(ccA_inst.ins, ti.ins, True, "A-staging before A2A#1")

    ccB_inst = nc.gpsimd.collective_compute(
        "AllToAll", mybir.AluOpType.bypass,
        ins=[cc_inB[:]], outs=[cc_outB[:]],
        replica_groups=[list(range(N_CORES))],
    )
    for ti in stagingB:
        add_dep_helper(ccB_inst.ins, ti.ins, True, "B-staging before A2A#2")

    ph.close()

    opool = ctx.enter_context(tc.tile_pool(name="ops", bufs=2, space="PSUM"))
    for tt in range(QB // 128):
        emit_proj_tile(cc_outB, tt, QB, opool, "o")


_COMPILED_NC = None


def _get_nc():
    global _COMPILED_NC
    if _COMPILED_NC is None:
        nc = bacc.Bacc("TRN2", target_bir_lowering=False, debug=False,
                       num_devices=N_CORES)
        build_program(nc)
        nc.compile()
        _COMPILED_NC = nc
    return _COMPILED_NC


def kernel(x, W_attn, b_attn, W_proj, b_proj):
    global LAST_RESULTS
    nc = _get_nc()

    bf = ml_dtypes.bfloat16
    xT_np = np.ascontiguousarray(
        np.asarray(x, np.float32).reshape(BT, C).T
    ).astype(bf)
    W_attn = np.asarray(W_attn, np.float32)
    b_attn = np.asarray(b_attn, np.float32)
    wp_np = np.asarray(W_proj, np.float32).astype(bf)
    bp_np = np.asarray(b_proj, np.float32)

    in_maps = []
    for c in range(N_CORES):
        s = slice(c * FPC, (c + 1) * FPC)
        in_maps.append({
            "xT": xT_np,
            "wq": np.ascontiguousarray(W_attn[:, s]).astype(bf),
            "wk": np.ascontiguousarray(W_attn[:, C:2 * C][:, s]).astype(bf),
            "wv": np.ascontiguousarray(W_attn[:, 2 * C:][:, s]).astype(bf),
            "bqkv": np.ascontiguousarray(
                np.stack([b_attn[s], b_attn[C:2 * C][s], b_attn[2 * C:][s]])
            ).astype(np.float32),
            "wp": wp_np,
            "bp": bp_np,
        })

    res = run_bass_kernel_spmd(nc, in_maps, core_ids=list(range(N_CORES)))
    LAST_RESULTS = res
    # Core r holds rows of blocks (b=r//4, qb=r%4) then (b=2+r//4, qb=r%4).
    full = np.empty((BT, C), np.float32)
    for b in range(B):
        for qb in range(NQB):
            r = (b % 2) * 4 + qb
            half = 0 if b < 2 else 1
            full[b * T + qb * QB : b * T + (qb + 1) * QB] = (
                res.results[r]["out"][half * QB : (half + 1) * QB]
            )
    return full.reshape(B, T, C)
